# revision 8
# baseline (speedup 1.0000x reference)
"""Bass/Trainium2 kernel for nn_BmmEnsemble (ensemble-of-MLPs atomic energy sum).

Sharding: 8 cores; core c owns species c//2, half c%2 (12500/2 = 6250 atoms).
Per (ensemble member e, 512-atom block pair) the core runs the 3-layer MLP
(1008->256->192->160, CELU alpha=0.1) in fp32r and reduces layer-3
activations to per-feature atom sums; layer 4 / ensemble mean / final sum
are linear and done on the host in fp64.

Measured-roofline notes: PE weight loads run at ~1 row/cycle and are only
hidden when compute >= load (N=512 moving, fp32r). fp8 DoubleRow would be
2x/instr but needs hi+lo split weights for the 2e-2 gate (systematic
rounding error), cancelling the gain, and is load-bound at N<=256 anyway.
So matmuls stay fp32r/exact and the win over the old kernel is a much
lighter elementwise path so PE never stalls on ACT/DVE:

  CELU: z' = z + b + alpha in PSUM (bias via K=2 bias-row matmul against
    a ones tile, zero extra ACT/DVE work); u = ACT Exp(10 z' + ln a - 1);
    g = one DVE stt max(z', min(u, alpha)) = celu(z+b) + alpha.
  L2 pad partitions are seeded 0.1 by the bias matmul so g2's pad rows
    are constant 0.1; w3's k-chunk1 row 64 = b3_adj/0.1 folds L3's bias.
  L3 sums via stt accum_out: C banks hold f0:128 x 512 atoms per (e);
    E banks pack f128:160 as four 32-partition stripes = 4 ensemble
    members of the same block pair (tile_position column offsets), so no
    PE/ACT/DVE lanes are wasted on the 32-feature tail.

Emission is software-pipelined: step t runs PE work in the order
[L2(t-1), L3(t-2), L1(t)] so single-buffered A/B PSUM banks are safe.
"""

import numpy as np

import concourse.bacc as bacc
import concourse.tile as tile
import concourse.mybir as mybir
from concourse.bass_utils import run_bass_kernel_spmd

F32 = mybir.dt.float32
F32R = mybir.dt.float32r
AF = mybir.ActivationFunctionType
ALU = mybir.AluOpType

S = 4
E = 8
N = 50000
AEV = 1008
ALPHA = 0.1
LNA1 = float(np.log(ALPHA) - 1.0)
NCORES = 8
NA = N // S // 2            # atoms per core: 6250
BP = 512                    # atoms per slot (block pair)
NBP = (NA + BP - 1) // BP   # 13 (12 x 512 + 106)
PADC = 0.1                  # constant seeded into L2 pad partitions
NCOL_C = NBP * E            # 104
NCOL = NCOL_C * 2           # + per-slot E cols: 208
NSLOT = NBP * E             # 104


def _build():
    nc = bacc.Bacc("TRN2", target_bir_lowering=False, debug=False,
                   num_devices=NCORES)

    xT = nc.dram_tensor("xT", [1024, NA], F32R, kind="ExternalInput")
    w1 = nc.dram_tensor("w1", [E, 1024, 256], F32R, kind="ExternalInput")
    w2 = nc.dram_tensor("w2", [E, 2, 128, 256], F32R, kind="ExternalInput")
    b2r = nc.dram_tensor("b2r", [E, 2, 256], F32R, kind="ExternalInput")
    w3 = nc.dram_tensor("w3", [E, 2, 128, 160], F32R, kind="ExternalInput")
    onesd = nc.dram_tensor("onesd", [2, 512], F32R, kind="ExternalInput")
    acc = nc.dram_tensor("acc", [128, NCOL], F32, kind="ExternalOutput")

    with tile.TileContext(nc) as tc:
        with (
            tc.tile_pool(name="wp", bufs=1) as wp,
            tc.tile_pool(name="xp", bufs=2) as xp,
            tc.tile_pool(name="up", bufs=2) as up,
            tc.tile_pool(name="gp", bufs=2) as gp,
            tc.tile_pool(name="ps", bufs=1, space="PSUM") as ps,
        ):
            # ---- resident weights ----
            w1t, w2t, b2t, w3t = {}, {}, {}, {}
            for e in range(E):
                for kc in range(8):
                    t = wp.tile([128, 256], F32R, tag=f"w1_{e}_{kc}")
                    nc.sync.dma_start(t[:], w1[e, kc * 128:(kc + 1) * 128])
                    w1t[e, kc] = t
                for kc in range(2):
                    t = wp.tile([128, 256], F32R, tag=f"w2_{e}_{kc}")
                    nc.sync.dma_start(t[:], w2[e, kc])
                    w2t[e, kc] = t
                    t = wp.tile([128, 160], F32R, tag=f"w3_{e}_{kc}")
                    nc.sync.dma_start(t[:], w3[e, kc])
                    w3t[e, kc] = t
                t = wp.tile([2, 256], F32R, tag=f"b2_{e}")
                nc.sync.dma_start(t[:], b2r[e])
                b2t[e] = t
            ones = wp.tile([2, 512], F32R, tag="ones")
            nc.sync.dma_start(ones[:], onesd[:])
            bexp = wp.tile([128, 1], F32, tag="bexp")
            nc.vector.memset(bexp[:], LNA1)
            acct = wp.tile([128, NCOL], F32, tag="acct")

            # ---- x prefetch ----
            xtiles = {}

            def emit_x_dma(bp):
                if bp >= NBP or bp in xtiles:
                    return
                na = min(BP, NA - bp * BP)
                lst = []
                for kc in range(8):
                    t = xp.tile([128, BP], F32R, tag=f"x{kc}")
                    nc.sync.dma_start(t[:, :na],
                                      xT[kc * 128:(kc + 1) * 128,
                                         bp * BP:bp * BP + na])
                    lst.append(t)
                xtiles[bp] = lst

            emit_x_dma(0)
            emit_x_dma(1)

            state = {}
            ctx = {}

            def slot(t):
                bp, e = divmod(t, E)
                return bp, e, min(BP, NA - bp * BP)

            def l1mm(t):
                bp, e, na = slot(t)
                if e == 0:
                    emit_x_dma(bp + 1)
                st = state.setdefault(t, {})
                st["A"] = []
                xt = xtiles[bp]
                for m in range(2):
                    A = ps.tile([128, 512], F32, tag=f"A{m}")
                    for kc in range(8):
                        nc.tensor.matmul(
                            A[:, :na],
                            w1t[e, kc][:, m * 128:(m + 1) * 128],
                            xt[kc][:, :na],
                            start=(kc == 0), stop=(kc == 7))
                    st["A"].append(A)

            def l1ew(t):
                bp, e, na = slot(t)
                st = state[t]
                g = gp.tile([128, 1024], F32R, tag="g1")
                for m in range(2):
                    A = st["A"][m]
                    u = up.tile([128, 512], F32, tag=f"u1{m}")
                    nc.scalar.activation(u[:, :na], A[:, :na], AF.Exp,
                                         bias=bexp[:, 0:1], scale=10.0)
                    nc.vector.scalar_tensor_tensor(
                        g[:, m * 512:m * 512 + na], u[:, :na], ALPHA,
                        A[:, :na], op0=ALU.min, op1=ALU.max)
                st["g1"] = g

            def l2mm(t):
                bp, e, na = slot(t)
                st = state[t]
                g1 = st["g1"]
                st["B"] = []
                for m in range(2):
                    B = ps.tile([128, 512], F32, tag=f"B{m}")
                    nc.tensor.matmul(B[:, :na],
                                     b2t[e][:, m * 128:(m + 1) * 128],
                                     ones[:, :na], start=True, stop=False)
                    for kc in range(2):
                        nc.tensor.matmul(
                            B[:, :na],
                            w2t[e, kc][:, m * 128:(m + 1) * 128],
                            g1[:, kc * 512:kc * 512 + na],
                            start=False, stop=(kc == 1))
                    st["B"].append(B)

            def l2ew(t):
                bp, e, na = slot(t)
                st = state[t]
                g = gp.tile([128, 1024], F32R, tag="g2")
                for m in range(2):
                    B = st["B"][m]
                    u = up.tile([128, 512], F32, tag=f"u2{m}")
                    nc.scalar.activation(u[:, :na], B[:, :na], AF.Exp,
                                         bias=bexp[:, 0:1], scale=10.0)
                    nc.vector.scalar_tensor_tensor(
                        g[:, m * 512:m * 512 + na], u[:, :na], ALPHA,
                        B[:, :na], op0=ALU.min, op1=ALU.max)
                st["g2"] = g

            def l3mm(t):
                bp, e, na = slot(t)
                st = state[t]
                g2 = st["g2"]
                C = ps.tile([128, 512], F32, tag="C", bufs=2, name=f"C_{t}")
                st["C"] = C
                Eb = ps.tile([32, 512], F32, tag="Eb", bufs=2, name=f"E_{t}")
                st["E"] = Eb
                for kc in range(2):
                    nc.tensor.matmul(C[:, :na],
                                     w3t[e, kc][:, 0:128],
                                     g2[:, kc * 512:kc * 512 + na],
                                     start=(kc == 0), stop=(kc == 1))
                for kc in range(2):
                    nc.tensor.matmul(Eb[:, :na],
                                     w3t[e, kc][:, 128:160],
                                     g2[:, kc * 512:kc * 512 + na],
                                     start=(kc == 0), stop=(kc == 1))

            def l3ew(t):
                bp, e, na = slot(t)
                st = state[t]
                C, Eb = st["C"], st["E"]
                u = up.tile([128, 512], F32, tag="uC")
                nc.scalar.activation(u[:, :na], C[:, :na], AF.Exp,
                                     bias=bexp[:, 0:1], scale=10.0)
                s3 = gp.tile([128, 512], F32, tag="s3")
                col = bp * E + e
                nc.vector.scalar_tensor_tensor(
                    s3[:, :na], u[:, :na], ALPHA, C[:, :na],
                    op0=ALU.min, op1=ALU.max,
                    accum_out=acct[:, col:col + 1])
                uE = up.tile([32, 512], F32, tag="uE")
                nc.scalar.activation(uE[:, :na], Eb[:, :na], AF.Exp,
                                     bias=bexp[:32, 0:1], scale=10.0)
                sE = gp.tile([32, 512], F32, tag="sE")
                col = NCOL_C + bp * E + e
                nc.vector.scalar_tensor_tensor(
                    sE[:, :na], uE[:, :na], ALPHA, Eb[:, :na],
                    op0=ALU.min, op1=ALU.max,
                    accum_out=acct[:32, col:col + 1])
                del state[t]

            for t in range(NSLOT + 2):
                if 1 <= t <= NSLOT:
                    l2mm(t - 1)
                if t >= 2:
                    l3mm(t - 2)
                if t < NSLOT:
                    l1mm(t)
                if 1 <= t <= NSLOT:
                    l2ew(t - 1)
                if t >= 2:
                    l3ew(t - 2)
                if t < NSLOT:
                    l1ew(t)

            nc.sync.dma_start(acc[:], acct[:])
    nc.compile()
    return nc


_NC = None


def _get_nc():
    global _NC
    if _NC is None:
        _NC = _build()
    return _NC


def _prep_inputs(inputs):
    aev = np.asarray(inputs["aev"], dtype=np.float32).reshape(N, AEV)
    idx = np.asarray(inputs["idx"])
    Ws = [np.asarray(inputs[f"W{i}"], dtype=np.float64) for i in (1, 2, 3, 4)]
    bs = [np.asarray(inputs[f"b{i}"], dtype=np.float64) for i in (1, 2, 3, 4)]

    in_maps = []
    for c in range(NCORES):
        s, h = c // 2, c % 2
        sel = np.asarray(idx[s, h * NA:(h + 1) * NA])
        xTc = np.zeros((1024, NA), dtype=np.float32)
        xTc[:AEV] = aev[sel].T
        xTc[AEV] = 1.0

        w1c = np.zeros((E, 1024, 256), dtype=np.float32)
        w1c[:, :AEV] = Ws[0][s]
        w1c[:, AEV] = bs[0][s][:, 0, :] + ALPHA

        w2c = np.zeros((E, 2, 128, 256), dtype=np.float32)
        w2c[:, :, :, :192] = Ws[1][s].reshape(E, 2, 128, 192)

        b2c = np.zeros((E, 2, 256), dtype=np.float32)
        b2a = np.full((E, 256), PADC)
        b2a[:, :192] = bs[1][s][:, 0, :] - ALPHA * Ws[1][s].sum(axis=1) + ALPHA
        b2c[:, 0, :] = b2a.astype(np.float32)

        w3c = np.zeros((E, 2, 128, 160), dtype=np.float32)
        w3c[:, 0] = Ws[2][s][:, 0:128]
        w3c[:, 1, 0:64] = Ws[2][s][:, 128:192]
        b3a = bs[2][s][:, 0, :] - ALPHA * Ws[2][s].sum(axis=1) + ALPHA
        w3c[:, 1, 64] = (b3a / PADC).astype(np.float32)

        in_maps.append({"xT": xTc, "w1": w1c, "w2": w2c, "b2r": b2c,
                        "w3": w3c, "onesd": np.ones((2, 512), np.float32)})
    return in_maps, Ws, bs


def _finish(results, Ws, bs):
    W4, b4 = Ws[3], bs[3]  # [S,E,160,1], [S,E,1,1]
    total = 0.0
    for c in range(NCORES):
        s = c // 2
        a = results[c]["acc"].astype(np.float64)  # [128, NCOL]
        for e in range(E):
            g3 = np.zeros(160)
            cols = [bp * E + e for bp in range(NBP)]
            g3[0:128] = a[:, cols].sum(axis=1)
            ecols = [NCOL_C + bp * E + e for bp in range(NBP)]
            g3[128:160] = a[0:32, ecols].sum(axis=1)
            h3 = g3 - ALPHA * NA
            total += (h3 @ W4[s, e, :, 0] + NA * b4[s, e, 0, 0]) / E
    return np.array([total], dtype=np.float32)


def _run(inputs, **spmd_kwargs):
    in_maps, Ws, bs = _prep_inputs(inputs)
    nc = _get_nc()
    res = run_bass_kernel_spmd(nc, in_maps, list(range(NCORES)), **spmd_kwargs)
    return _finish(res.results, Ws, bs), res


def kernel(**inputs) -> np.ndarray:
    out, _ = _run(inputs)
    return out
